# revision 57
# baseline (speedup 1.0000x reference)
"""Trainium2 Bass kernel for nn_Attention_82660940579436.

Computation (see reference):
    q     = mean_s(hidden @ Wq.T + bq)            [B, H]
    key   = tanh(hidden @ Wk.T + bk)              [S, B, H]
    score = einsum('bsh,bh->bs', key, q) + mask   [B, S]
    out   = softmax(score) @ key                  [B, H]

Sharding: data-parallel over batch. B=32 over 8 cores -> 4 batches/core.

v6 design (290us baseline -> 250 -> 224 -> this):
  - Hidden is pre-transposed and bf16-cast ON HOST into the exact SBUF
    layout the key matmuls need ([chunk, j_local, (tile, cj, s)]), DMA'd
    in 8-tile chunks (SWDGE descriptor-gen costs ~1us of Pool time per
    load, so per-tile loads are out).
  - One batch per 128-token tile: 4 groups of 32 tiles, tok = s_local.
  - MASKED-TILE SKIPPING: positions s >= length only reach the output
    through exp(-60) ~ 1e-26, so tiles entirely past a batch's length
    need no key projection / softmax work.  Only q = mean_s(...) needs
    the full sequence.  Batches are SORTED by length on the host and
    assigned to (core, group) slots so each group's max length across
    the 8 cores (the SPMD program is shared) is minimal; the program is
    compiled per set of group tile-counts (cached).  For uniform random
    lengths this skips ~40% of the expensive work.
  - PREPASS/KEYPASS split: a cheap prepass (chunk DMA + per-chunk sums
    via DVE TSP-accumulate + the tiny q chain) runs one group ahead,
    interleaved into the previous group's keypass, so q(g) is ready
    when keypass(g) starts and phase B (softmax) of each tile follows
    its key projection immediately.  Phase A (PE-bound) and phase B
    (DVE/ACT/Pool-bound) overlap everywhere except a short head.
    Chunks are re-loaded for the keypass (DMA device has headroom;
    SBUF does not - keys alone take up to 128KiB/partition).
  - Work splits: bias bk via PE rank-1 matmul; score mul+rowsum fused
    in one DVE scalar_tensor_tensor (TSP-reduce is DVE-only) with a
    per-window fraction diverted to GPSIMD-mul + ACT-accumulate where
    the prepass loads the DVE.

exp() needs no max-subtraction: scores are O(1) by construction, masked
positions get -60 bias -> exp underflows to ~1e-27 (reference's -10000
mask likewise produces exact zeros after its own softmax).

All constants ship in two packed tensors (one fp32, one bf16) loaded by a
single DMA each; two dummy PE ops observe those DMA lanes up front so no
real matmul needs two sync-waits (walrus allows one on a Matmult).
"""

import sys
from contextlib import ExitStack

import numpy as np

if "/opt/trn_rl_repo" not in sys.path:
    sys.path.insert(0, "/opt/trn_rl_repo")

import ml_dtypes  # noqa: E402

import concourse.bacc as bacc  # noqa: E402
import concourse.bass as bass  # noqa: E402
import concourse.mybir as mybir  # noqa: E402
import concourse.tile as tile  # noqa: E402
from concourse.bass_utils import run_bass_kernel_spmd  # noqa: E402

S, B, H = 4096, 32, 512
NCORES = 8
BPC = B // NCORES  # 4 batches per core = 4 groups
NT = 128  # tiles per core
TPG = NT // BPC  # 32 tiles per group
TOK = S // TPG  # 128 tokens (s-positions) per tile
HC = H // 128  # 4 chunks of the H (j / i) dims
CHUNK = 8  # tiles per hidden DMA
NCHUNK = NT // CHUNK
CPG = TPG // CHUNK  # chunks per group
MASK_NEG = -60.0
F32 = mybir.dt.float32
BF16 = mybir.dt.bfloat16
AF = mybir.ActivationFunctionType
ALU = mybir.AluOpType
BF16NP = ml_dtypes.bfloat16

# fp32 const pack layout (offsets in fp32 elements, [128, PACKF] tensor)
OFF_MASK = 0  # [128, NT] mask bias (0 / MASK_NEG), col=global tile
OFF_BQ = 128  # [1, 512] bq row
OFF_ZERO = 640  # [128, 1] zeros (tanh bias)
PACKF = 641
# bf16 const pack layout ([128, PACKB]) — matmul operands live here:
# fp32 matmuls run at 1/4 rate on TRN2, bf16 at full rate.  The pack is
# DMA'd in two pieces: an urgent prefix (everything the first key chains
# need) and a lazy tail (Wq & co, needed ~15 tiles in), so the first
# hidden chunk isn't stuck behind a 3.7us const transfer.
OFFB_WK = 0  # [128, 2048] WkT chunks
OFFB_BK = 2048  # [1, 512] bk on partition 0 (PE bias matmul rhs)
OFFB_ONESROW = 2560  # [1, 128] ones on partition 0 (PE bias/bcast lhsT)
OFFB_ONES = 2688  # [128, 1] ones
OFFB_OH = 2690  # [128, 4*4] one-hot rows: cols g*4+g' = (g == g')
PACKB_URG = 2706  # end of the urgent prefix
OFFB_WQ = 2706  # [128, 2048] WqT chunks
OFFB_BKB = 4754  # [128, 512] bk broadcast (DVE in-place PSUM bias add)
PACKB = 5266

# tuning knobs (read at build time)
KNOBS = {
    "hp_bufs": 2,  # prepass chunk buffers
    "hk_bufs": 4,  # keypass chunk buffers
    "keyps_bufs": 5,
    "small_bufs": 6,
    "stagger": 5,  # tiles of keypass(g) emitted before B(g) starts
    # phase B: fraction (in tenths) of tiles whose fused score mul+rowsum
    # runs on DVE (scalar_tensor_tensor), per window; the rest go to
    # GPSIMD-mul + ACT-accumulate.  Window 2 carries the heaviest prepass
    # red load on DVE; window 3 has no prepass.
    "stt_dve10": [7, 7, 8, 9],
    # fraction of tiles whose bk bias is added by a DVE in-place PSUM add
    # instead of the PE rank-1 matmul (trades 213ns of PE for 655ns of DVE)
    "bias_dve": lambda t: False,
}


def _build_kernel_body(tc, aps, t_cnts):
    nc = tc.nc
    x, packf, packb, y = aps["x"], aps["packf"], aps["packb"], aps["y"]

    with ExitStack() as ctx:
        consts = ctx.enter_context(tc.tile_pool(name="consts", bufs=1))
        php = ctx.enter_context(tc.tile_pool(name="hp", bufs=KNOBS["hp_bufs"]))
        phk = ctx.enter_context(tc.tile_pool(name="hk", bufs=KNOBS["hk_bufs"]))
        # Keys are consumed by B a handful of tiles after production (global
        # backlog lag peaks at ~q(0)'s landing spot, ~26 tiles); a 40-tile
        # ring bounds SBUF for ANY lengths (worst case sum(t_cnts)=128 would
        # otherwise need 128KiB/partition and overflow SBUF).
        pkeys = ctx.enter_context(
            tc.tile_pool(name="keys", bufs=min(max(1, sum(t_cnts)), 40))
        )
        psmall = ctx.enter_context(tc.tile_pool(name="small", bufs=KNOBS["small_bufs"]))
        pacc = ctx.enter_context(tc.tile_pool(name="acc", bufs=1))
        pq = ctx.enter_context(tc.tile_pool(name="q", bufs=2))
        pps_key = ctx.enter_context(
            tc.tile_pool(name="ps_key", bufs=KNOBS["keyps_bufs"], space="PSUM")
        )
        pps_acc = ctx.enter_context(tc.tile_pool(name="ps_acc", bufs=1, space="PSUM"))
        pps_sm = ctx.enter_context(tc.tile_pool(name="ps_sm", bufs=1, space="PSUM"))

        # ---- constants.  Urgent cb prefix first: it gates the first matmul
        # chain; cb's lazy tail and cf (mask/bq/zero) are loaded after the
        # first hidden chunk so they don't cut ahead of it on the DMA device
        cb = consts.tile([128, PACKB], BF16)
        nc.sync.dma_start(cb[:, :PACKB_URG], packb[:, :PACKB_URG])
        cf = consts.tile([128, PACKF], F32)

        def wk_sb(c):
            return cb[:, OFFB_WK + c * 512 : OFFB_WK + (c + 1) * 512]

        def wq_sb(c):
            return cb[:, OFFB_WQ + c * 512 : OFFB_WQ + (c + 1) * 512]

        maskb_sb = cf[:, OFF_MASK : OFF_MASK + NT]
        bq_sb = cf[0:1, OFF_BQ : OFF_BQ + H]
        zero_sb = cf[:, OFF_ZERO : OFF_ZERO + 1]
        bk_sb = cb[0:1, OFFB_BK : OFFB_BK + H]
        ones_row_sb = cb[0:1, OFFB_ONESROW : OFFB_ONESROW + 128]
        ones1_sb = cb[:, OFFB_ONES : OFFB_ONES + 1]

        def oh_sb(g):
            return cb[:, OFFB_OH + g * BPC : OFFB_OH + (g + 1) * BPC]

        bkb_sb = cb[:, OFFB_BKB : OFFB_BKB + H]

        # Dummy PE op: observe the cb DMA lane once, so no real matmul ever
        # needs two sync-waits (walrus S3_LW limit is one).  cf's observer
        # is emitted after its (delayed) DMA below.
        scr = pps_sm.tile([128, H], F32, tag="sm")
        nc.tensor.matmul(scr[0:1, 0:1], ones1_sb, ones1_sb, start=True, stop=True)

        # per-tile partial sums of h: red_all[j, T*HC + c] = sum_s hT (fp32,
        # written by the prepass TSP accumulators; reduced per group in emit_q)
        red_all = pacc.tile([128, NT * HC], F32)

        keys = [None] * NT
        qrep = [None] * BPC
        numer_ps = pps_acc.tile([BPC, H], F32, tag="numer")
        den_ps = pps_acc.tile([BPC, 1], F32, tag="den")
        nb_total = sum(t_cnts)
        nb_done = 0  # emitted B tiles, to set start/stop on the accumulators

        hp_cur = [None]  # current prepass chunk tile

        def emit_pre_dma(ch):
            h_t = php.tile([128, CHUNK * H], BF16, tag="hp")
            nc.gpsimd.dma_start(h_t, x[ch])
            hp_cur[0] = h_t

        def emit_red(g, t):
            T = g * TPG + t
            hview = hp_cur[0][:, (t % CHUNK) * H : (t % CHUNK + 1) * H]
            for c in range(HC):
                pd = psmall.tile([128, 128], BF16, tag="pd")
                nc.vector.tensor_scalar(
                    pd,
                    hview[:, c * 128 : (c + 1) * 128],
                    1.0,
                    0.0,
                    op0=ALU.mult,
                    op1=ALU.add,
                    accum_out=red_all[:, T * HC + c : T * HC + c + 1],
                )

        def emit_q(g):
            redg = pacc.tile([128, HC], F32, tag=f"rg{g}")
            nc.vector.tensor_reduce(
                redg,
                red_all[
                    :, g * TPG * HC : (g + 1) * TPG * HC
                ].rearrange("p (t c) -> p c t", t=TPG, c=HC),
                axis=mybir.AxisListType.X,
                op=ALU.add,
            )
            maccb = pacc.tile([128, HC], BF16, tag=f"mb{g}")
            nc.vector.tensor_copy(maccb, redg)
            q_ps = pps_sm.tile([128, H], F32, tag="sm")
            for c in range(HC):
                nc.tensor.matmul(
                    q_ps[0:1, :],
                    maccb[:, c : c + 1],
                    wq_sb(c),
                    start=(c == 0),
                    stop=(c == HC - 1),
                )
            q_sb = pq.tile([1, H], F32, tag="q")
            nc.scalar.mul(q_sb, q_ps[0:1, :], 1.0 / S)
            q_b = pq.tile([1, H], BF16, tag="qb")
            nc.vector.tensor_add(q_b, q_sb, bq_sb)
            qrep_ps = pps_sm.tile([128, H], F32, tag="sm")
            nc.tensor.matmul(qrep_ps, ones_row_sb, q_b, start=True, stop=True)
            qrep_g = pq.tile([128, H], BF16, tag="qr")
            nc.vector.tensor_copy(qrep_g, qrep_ps)
            qrep[g] = qrep_g

        hk_tiles = {}

        def emit_hk_dma(g, t):
            # load only the tiles this group actually computes
            T = g * TPG + t
            ntl = min(CHUNK, t_cnts[g] - t)
            h_t = phk.tile([128, CHUNK * H], BF16, tag="hk")
            if T == 0 and ntl > 2:
                # split the very first load so tile 0's matmuls aren't gated
                # on the whole chunk crossing the DMA device
                nc.gpsimd.dma_start(h_t[:, : 2 * H], x[0][:, : 2 * H])
                nc.gpsimd.dma_start(h_t[:, 2 * H : ntl * H], x[0][:, 2 * H : ntl * H])
            else:
                nc.gpsimd.dma_start(h_t[:, : ntl * H], x[T // CHUNK][:, : ntl * H])
            hk_tiles[(g, t // CHUNK)] = h_t

        def emit_key(g, t):
            T = g * TPG + t
            if t % CHUNK == 0 and (g, t // CHUNK) not in hk_tiles:
                emit_hk_dma(g, t)
            hview = hk_tiles[(g, t // CHUNK)][:, (t % CHUNK) * H : (t % CHUNK + 1) * H]
            key_ps = pps_key.tile([TOK, H], F32, tag="key")
            bias_dve = KNOBS["bias_dve"](T)
            if not bias_dve:
                nc.tensor.matmul(key_ps, ones_row_sb, bk_sb, start=True, stop=False)
            for c in range(HC):
                nc.tensor.matmul(
                    key_ps,
                    hview[:, c * 128 : (c + 1) * 128],
                    wk_sb(c),
                    start=(c == 0 and bias_dve),
                    stop=(c == HC - 1),
                )
            if bias_dve:
                nc.vector.tensor_add(key_ps, key_ps, bkb_sb)
            key_t = pkeys.tile([TOK, H], BF16, tag="key")
            nc.scalar.activation(key_t, key_ps, AF.Tanh, bias=zero_sb)
            keys[T] = key_t

        def emit_b(g, t):
            nonlocal nb_done
            T = g * TPG + t
            key_t = keys[T]
            sc_t = psmall.tile([TOK, 1], F32, tag="sc")
            if (t * 7) % 10 < KNOBS["stt_dve10"][g]:
                prod = psmall.tile([TOK, H], BF16, tag="prod")
                nc.vector.scalar_tensor_tensor(
                    prod, key_t, 1.0, qrep[g], ALU.mult, ALU.mult, accum_out=sc_t
                )
            else:
                prod = psmall.tile([TOK, H], BF16, tag="prodg")
                nc.gpsimd.tensor_mul(prod, key_t, qrep[g])
                pc = psmall.tile([TOK, H], BF16, tag="pc")
                nc.scalar.activation(pc, prod, AF.Copy, accum_out=sc_t)
            e_b = psmall.tile([TOK, 1], F32, tag="e")
            nc.scalar.activation(e_b, sc_t, AF.Exp, bias=maskb_sb[:, T : T + 1])
            # ei[s, g'] = e[s] * (g' == g): lets one [4, H] PSUM accumulator
            # collect all four groups (matmul outs must start at partition 0)
            ei_t = psmall.tile([TOK, BPC], BF16, tag="ei")
            nc.vector.tensor_scalar_mul(ei_t, oh_sb(g), e_b)
            first = nb_done == 0
            last = nb_done == nb_total - 1
            nc.tensor.matmul(numer_ps, ei_t, key_t, start=first, stop=last)
            nc.tensor.matmul(den_ps, ei_t, ones1_sb, start=first, stop=last)
            nb_done += 1

        # ---- windows: keypass(g) || B(g) || prepass(g+1) ----
        # The first keypass chunk is loaded before anything else so PE can
        # start immediately; prepass(0) and q(0) interleave into window 0
        # (B(0) is gated on q(0)); each window prefetches the next window's
        # first keypass chunk so PE never waits at a window boundary.
        # cf is only read by ACT (tanh/exp bias) and DVE (bq add) — PE never
        # touches it, so no PE observer dummy is needed for its DMA lane.
        emit_hk_dma(0, 0)
        nc.sync.dma_start(cb[:, PACKB_URG:], packb[:, PACKB_URG:])
        nc.sync.dma_start(cf, packf)

        def emit_lazy_observer():
            # PE observer for the lazy cb lane (scheduled ~10 tiles in, well
            # after the transfer lands; keeps the q matmuls at one wait each)
            nc.tensor.matmul(
                scr[0:1, 0:1], wq_sb(0)[0:1, 0:1], wq_sb(0)[0:1, 0:1],
                start=True, stop=True,
            )

        q_done = [False] * BPC

        def prepass_ops(gn, with_q):
            ops = []
            for ch in range(CPG):
                cg = gn * CPG + ch
                ops.append(lambda cg=cg: emit_pre_dma(cg))
                for t in range(ch * CHUNK, (ch + 1) * CHUNK):
                    ops.append(lambda gn=gn, t=t: emit_red(gn, t))
            if with_q:
                def qop(gn=gn):
                    emit_q(gn)
                    q_done[gn] = True
                ops.append(qop)
            return ops

        # One GLOBAL pre-op stream — prepass(0..3), each ending in its q, with
        # next-window hk prefetches dropped mid-prepass — spread across all
        # keypass tiles proportionally and front-loaded (done by ~80%), so
        # each q lands roughly a window early and the short late windows
        # carry no prepass load on DVE.
        stream = prepass_ops(0, True)
        stream.insert(30, emit_lazy_observer)
        for gn in range(1, BPC):
            ops = prepass_ops(gn, True)
            ops.insert(len(ops) // 2, lambda gn=gn: emit_hk_dma(gn, 0))
            stream += ops
        nstream = len(stream)
        ntiles = sum(t_cnts)
        gt = 0  # global keypass tile counter
        done = 0

        # B ops accumulate into one PSUM chain, so their order is free: keep a
        # global backlog and drain it lazily (up to 3 per keypass tile), so a
        # window's tail B work overlaps the next window's keypass instead of
        # stalling PE at the boundary.
        stg = KNOBS["stagger"]
        backlog = []
        for g in range(BPC):
            m = t_cnts[g]
            stg_g = min(stg, max(2, m // 2))
            for t in range(m):
                emit_key(g, t)
                gt += 1
                backlog.append((g, t))
                # prefetch the next intra-window chunk two tiles into this one
                if t % CHUNK == 2 and t + CHUNK - 2 < m:
                    nxt_t = (t // CHUNK + 1) * CHUNK
                    if (g, nxt_t // CHUNK) not in hk_tiles:
                        emit_hk_dma(g, nxt_t)
                want = min(nstream, nstream * gt * 2 // (1 * ntiles))
                while done < want:
                    stream[done]()
                    done += 1
                nb = 0
                while backlog and nb < 3:
                    bg, bt = backlog[0]
                    # keep `stg_g` tiles of slack behind the key production
                    # of the CURRENT window; older windows' tiles are free
                    if bg == g and bt > t - stg_g:
                        break
                    if not q_done[bg]:
                        break
                    backlog.pop(0)
                    emit_b(bg, bt)
                    nb += 1
        while done < nstream:
            stream[done]()
            done += 1
        for bg, bt in backlog:
            emit_b(bg, bt)

        # ---- out = numer / den ----
        rcp = pacc.tile([BPC, 1], F32)
        nc.vector.reciprocal(rcp, den_ps)
        out_sb = pacc.tile([BPC, H], F32)
        nc.vector.tensor_scalar_mul(out_sb, numer_ps, rcp)
        nc.sync.dma_start(y, out_sb)


_CACHE = {}


def _get_program(t_cnts=None):
    if t_cnts is None:
        t_cnts = _CACHE.get("last")
        assert t_cnts is not None, "no program built yet"
    t_cnts = tuple(int(t) for t in t_cnts)
    if t_cnts in _CACHE:
        _CACHE["last"] = t_cnts
        return _CACHE[t_cnts]
    nc = bacc.Bacc(None, target_bir_lowering=False, debug=False)
    aps = {
        "x": nc.dram_tensor(
            "x", [NCHUNK, 128, CHUNK * H], BF16, kind="ExternalInput"
        ).ap(),
        "packf": nc.dram_tensor("packf", [128, PACKF], F32, kind="ExternalInput").ap(),
        "packb": nc.dram_tensor("packb", [128, PACKB], BF16, kind="ExternalInput").ap(),
        "y": nc.dram_tensor("y", [BPC, H], F32, kind="ExternalOutput").ap(),
    }
    with tile.TileContext(nc) as tc:
        _build_kernel_body(tc, aps, t_cnts)
    nc.finalize()  # Bacc.compile: wait legalization (EVSEM splits), LDW moves
    _CACHE[t_cnts] = (nc, aps)
    _CACHE["last"] = t_cnts
    return nc, aps


def _plan(lengths):
    """Sort batches by length (desc); rank r -> (core r%8, group r//8).
    Returns (order, t_cnts): order[r] = original batch index; t_cnts[g] =
    tiles of keypass/B work for group g (max over cores, SPMD-shared)."""
    lens = np.asarray(lengths).astype(np.int64)
    order = np.argsort(-lens, kind="stable")
    sl = lens[order].reshape(BPC, NCORES)  # [group, core]
    t_cnts = np.ceil(sl.max(axis=1) / TOK).astype(int)
    return order, tuple(int(t) for t in t_cnts)


def _make_in_maps(hidden_states, Wq, bq, Wk, bk, lengths, order):
    hidden = np.asarray(hidden_states, dtype=np.float32)
    Wq = np.asarray(Wq, dtype=np.float32)
    Wk = np.asarray(Wk, dtype=np.float32)
    bqv = np.asarray(bq, dtype=np.float32)
    bkv = np.asarray(bk, dtype=np.float32)
    lens = np.asarray(lengths).astype(np.int64)

    p = np.arange(128)
    packb = np.zeros((128, PACKB), dtype=BF16NP)
    packb[:, OFFB_WK : OFFB_WK + 2048] = (
        np.ascontiguousarray(Wk.T)
        .reshape(HC, 128, H)
        .transpose(1, 0, 2)
        .reshape(128, 2048)
        .astype(BF16NP)
    )
    packb[:, OFFB_WQ : OFFB_WQ + 2048] = (
        np.ascontiguousarray(Wq.T)
        .reshape(HC, 128, H)
        .transpose(1, 0, 2)
        .reshape(128, 2048)
        .astype(BF16NP)
    )
    packb[0, OFFB_BK : OFFB_BK + H] = bkv.astype(BF16NP)
    packb[0, OFFB_ONESROW : OFFB_ONESROW + 128] = BF16NP(1.0)
    packb[:, OFFB_ONES] = BF16NP(1.0)
    for g in range(BPC):
        packb[:, OFFB_OH + g * BPC + g] = BF16NP(1.0)
    packb[:, OFFB_BKB : OFFB_BKB + H] = bkv[None, :].astype(BF16NP)

    base_packf = np.zeros((128, PACKF), dtype=np.float32)
    base_packf[0, OFF_BQ : OFF_BQ + H] = bqv

    in_maps = []
    t_idx = np.arange(NT)
    for c in range(NCORES):
        bsel = [int(order[g * NCORES + c]) for g in range(BPC)]
        hc = hidden[:, bsel, :]  # [S, 4, 512]
        # -> [chunk, j_local, (tile_in_chunk, cj, s_local)] bf16
        xc = np.ascontiguousarray(
            hc.transpose(1, 0, 2)  # [g, S, H]
            .reshape(BPC, CPG, CHUNK, TOK, HC, 128)  # g, ch, tl, p, cj, j
            .transpose(0, 1, 5, 2, 4, 3)  # g, ch, j, tl, cj, p
        ).reshape(NCHUNK, 128, CHUNK * H).astype(BF16NP)
        packf = base_packf.copy()
        b_of_t = np.array([bsel[g] for g in t_idx // TPG])  # [NT]
        s_full = (t_idx % TPG)[None, :] * TOK + p[:, None]  # [128, NT]
        valid = s_full < lens[b_of_t][None, :]
        packf[:, OFF_MASK : OFF_MASK + NT] = np.where(valid, 0.0, MASK_NEG)
        in_maps.append({"x": xc, "packf": packf, "packb": packb})
    return in_maps


def run(hidden_states, Wq, bq, Wk, bk, lengths, trace=False):
    """Run on 8 cores; returns (output [B, H] fp32, BassKernelResults)."""
    order, t_cnts = _plan(lengths)
    nc, _ = _get_program(t_cnts)
    in_maps = _make_in_maps(hidden_states, Wq, bq, Wk, bk, lengths, order)
    res = run_bass_kernel_spmd(
        nc, in_maps, core_ids=list(range(NCORES)), trace=trace
    )
    rows = np.concatenate([np.asarray(r["y"]) for r in res.results], axis=0)
    out = np.empty((B, H), dtype=np.float32)
    for c in range(NCORES):
        for g in range(BPC):
            out[int(order[g * NCORES + c])] = rows[c * BPC + g]
    return out, res


def kernel(hidden_states, Wq, bq, Wk, bk, lengths):
    out, _ = run(hidden_states, Wq, bq, Wk, bk, lengths)
    return out
